# revision 1
# baseline (speedup 1.0000x reference)
"""CapsNet (nn_CapsNet_25194278158439) Trainium2 Bass kernel, 8-core SPMD.

Math (faithful to the reference, including its W-tiling quirk):
  conv1 (9x9 valid) + relu -> conv2 (9x9 stride2) + bias -> primary capsules
  prim[b, p, d],  p = t*576 + s  (t = capsule type 0..31, s = spatial 0..575)
  u_hat[b,p,c,:] = prim[b,p,:] @ W[p mod 32]   (jnp.tile => weight index = s mod 32)
  3 rounds of routing-by-agreement over C=276 classes; out = ||v||  [2, 276].

Key factorization: u_hat is never materialized.  With g = s mod 32,
  s_vec[b,c,e]  = sum_g sum_d m[b,g,c,d] * W[g,c,d,e],
  m[b,g,c,d]    = sum_{p in g} softmax_c(b_log)[b,p,c] * prim[b,p,d]   (matmul)
  b_log update  = prim @ (W[g] @ v)                                    (matmul)

Sharding (8 cores):
  conv2 partial units (b, oc_half, ic_half); partials summed with one 1.2MB
  AllReduce of the full h2.  Routing shards rows by weight group: core k owns
  groups 4k..4k+3 (all types/batches), i.e. 4608 rows of 276 classes.  Each
  routing iteration does one 35KB AllReduce of the class vote s.
Per-core data (weight slices, gather offsets) is fed via per-core inputs so
all cores run one SPMD program; core-dependent addresses use sync-engine
registers loaded from the `gmeta` input.
"""

import os
import numpy as np

NC = 8          # cores
C = 276         # classes
D = 8           # primary capsule dim
E = 16          # digit capsule dim
NT = 32         # capsule types
S = 576         # spatial positions per type (24*24)
GL = 4          # weight groups per core
Q = 8           # row blocks per core: (batch, local group)
PCH = 5         # 128-row chunks per 640-padded block
CP = 384        # class dim padded to 3*128
CCH = 3         # class chunks

_CACHE = {}


def _build_program():
    import concourse.bass as bass
    import concourse.mybir as mybir
    import concourse.tile as tile
    from concourse import bacc
    from concourse.bass import ds
    from concourse.masks import make_identity

    f32 = mybir.dt.float32
    f32r = mybir.dt.float32r
    i32 = mybir.dt.int32
    AX = mybir.AxisListType
    AF = mybir.ActivationFunctionType
    ALU = mybir.AluOpType

    nc = bacc.Bacc("TRN2", target_bir_lowering=False, debug=False,
                   num_devices=NC)

    # ---- kernel I/O -------------------------------------------------------
    xb = nc.dram_tensor("xb", [64, 64], f32r, kind="ExternalInput").ap()
    w1T = nc.dram_tensor("w1T", [81, 128], f32r, kind="ExternalInput").ap()
    b1 = nc.dram_tensor("b1", [128, 1], f32, kind="ExternalInput").ap()
    w2T = nc.dram_tensor("w2T", [128, 81 * 128], f32r, kind="ExternalInput").ap()
    bias8 = nc.dram_tensor("bias8", [8, Q * S], f32, kind="ExternalInput").ap()
    gmeta = nc.dram_tensor("gmeta", [1, 16], i32, kind="ExternalInput").ap()
    Wcf = nc.dram_tensor("Wcf", [128, CCH * E * GL * D], f32,
                         kind="ExternalInput").ap()
    Wcf2 = nc.dram_tensor("Wcf2", [128, CCH * GL * D * E], f32,
                          kind="ExternalInput").ap()
    out = nc.dram_tensor("out", [2, C], f32, kind="ExternalOutput").ap()

    with tile.TileContext(nc) as tc:
        import contextlib
        with contextlib.ExitStack() as ctx:
            pool = ctx.enter_context(tc.tile_pool(name="const", bufs=1))
            dram = ctx.enter_context(tc.tile_pool(name="dram", bufs=1,
                                                  space="DRAM"))

            ident = pool.tile([128, 128], f32, tag="ident")
            make_identity(nc, ident[:])
            gmeta_sb = pool.tile([1, 16], i32, tag="gmeta")
            nc.sync.dma_start(gmeta_sb[:], gmeta)
            epsc = pool.tile([128, 1], f32, tag="epsc")
            zeroc = pool.tile([128, 1], f32, tag="zeroc")
            nc.vector.memset(epsc[:], 1e-8)
            nc.vector.memset(zeroc[:], 0.0)

            # collective bounce buffers
            ag_in = dram.tile([128, 576], f32, tag="ag_in")
            ag_out = dram.tile([8 * 128, 576], f32, tag="ag_out")
            s_in = dram.tile([128, 96], f32, tag="s_in")
            s_out = dram.tile([128, 96], f32, tag="s_out")

            # ============ conv phase ======================================
            with contextlib.ExitStack() as cctx:
                cpool = cctx.enter_context(tc.tile_pool(name="conv", bufs=1))
                ps1 = cctx.enter_context(
                    tc.tile_pool(name="ps1", bufs=2, space="PSUM"))
                ps2 = cctx.enter_context(
                    tc.tile_pool(name="ps2", bufs=1, space="PSUM"))

                w1T_sb = cpool.tile([81, 128], f32r, tag="w1T")
                b1_sb = cpool.tile([128, 1], f32, tag="b1")
                patches = cpool.tile([81, 3136], f32r, tag="patches")
                h1 = cpool.tile([128, 3136], f32r, tag="h1")
                w2T_sb = cpool.tile([128, 81 * 128], f32r, tag="w2T")
                h2p = cpool.tile([128, 576], f32, tag="h2p")

                nc.sync.dma_start(w1T_sb[:], w1T)
                nc.sync.dma_start(b1_sb[:], b1)

                # conv1 im2col: patches[(kh,kw), (oh,ow)] = x[oh+kh, ow+kw]
                for kh in range(9):
                    src = bass.AP(tensor=xb.tensor, offset=kh * 64,
                                  ap=[[1, 9], [64, 56], [1, 56]])
                    nc.sync.dma_start(
                        patches[kh * 9:(kh + 1) * 9, :].rearrange(
                            "p (a b) -> p a b", a=56), src)

                # conv2 weights: 4 chunked DMAs so HWDGE queues overlap
                for ci in range(4):
                    nc.sync.dma_start(
                        w2T_sb[:, ci * 2592: (ci + 1) * 2592],
                        w2T[:, ci * 2592: (ci + 1) * 2592])

                # conv1: h1[oc, s] = relu(w1.T @ patches + b1)
                for j in range(7):
                    pt = ps1.tile([128, 448], f32, tag="c1")
                    nc.tensor.matmul(pt[:], w1T_sb[:],
                                     patches[:, j * 448:(j + 1) * 448],
                                     start=True, stop=True)
                    nc.scalar.activation(h1[:, j * 448:(j + 1) * 448], pt[:],
                                         AF.Relu, bias=b1_sb[:, 0:1])

                # conv2: 81-position accumulation, stride 2
                psA = ps2.tile([128, 288], f32, tag="psA")
                psB = ps2.tile([128, 288], f32, tag="psB")
                hv = h1[:].rearrange("p (h w) -> p h w", w=56)
                for pos in range(81):
                    kh, kw = divmod(pos, 9)
                    vh = hv.rearrange("p (oh two) w -> p oh two w", two=2)[
                        :, kh // 2: kh // 2 + 24, kh % 2, :]
                    vw = vh.rearrange("p oh (ow two) -> p oh ow two", two=2)[
                        :, :, kw // 2: kw // 2 + 24, kw % 2]
                    lhsT = w2T_sb[:, pos * 128:(pos + 1) * 128]
                    nc.tensor.matmul(psA[:], lhsT, vw[:, 0:12, :],
                                     start=(pos == 0), stop=(pos == 80))
                    nc.tensor.matmul(psB[:], lhsT, vw[:, 12:24, :],
                                     start=(pos == 0), stop=(pos == 80))
                nc.scalar.copy(h2p[:, 0:288], psA[:])
                nc.scalar.copy(h2p[:, 288:576], psB[:])

                # permute columns group-major: h2p3[p, g*18+j] = h2p[p, j*32+g]
                h2p3 = cpool.tile([128, 576], f32, tag="h2p3")
                nc.vector.tensor_copy(
                    h2p3[:].rearrange("p (g j) -> p g j", g=32),
                    h2p[:].rearrange("p (j g) -> p g j", g=32))
                nc.sync.dma_start(ag_in[:], h2p3[:])

            nc.gpsimd.collective_compute(
                "AllGather", ALU.bypass,
                replica_groups=[list(range(NC))],
                ins=[ag_in[:].opt()], outs=[ag_out[:].opt()])

            # ============ routing phase ===================================
            with contextlib.ExitStack() as rctx:
                rp = rctx.enter_context(tc.tile_pool(name="rt", bufs=1))
                ps_t = rctx.enter_context(
                    tc.tile_pool(name="pst", bufs=1, space="PSUM"))
                ps_m = rctx.enter_context(
                    tc.tile_pool(name="psm", bufs=2, space="PSUM"))
                ps_u = rctx.enter_context(
                    tc.tile_pool(name="psu", bufs=1, space="PSUM"))

                rs = tc.alloc_tile_pool(name="rs", bufs=1)
                stage8 = rs.tile([8, Q * S], f32, tag="stage8")
                bias8_sb = rs.tile([8, Q * S], f32, tag="bias8")
                h2T = rp.tile([128, Q * PCH * D], f32, tag="h2T")
                tmp320 = rs.tile([128, Q * PCH * D], f32, tag="tmp320")
                sq = rp.tile([128, 40], f32, tag="sq")
                sp1 = rp.tile([128, 40], f32, tag="sp1")
                sp2 = rp.tile([128, 40], f32, tag="sp2")
                scale = rp.tile([128, 40], f32, tag="scale")
                sct = rs.tile([40, 128], f32, tag="sct")
                scale_flat = rs.tile([1, 5120], f32, tag="scale_flat")
                scale8 = rs.tile([8, 5120], f32, tag="scale8")
                primT = rp.tile([8, Q * 640], f32r, tag="primT")
                ps_sc = rp.tile([128, Q * PCH * D], f32r, tag="ps_sc")
                Zt = rp.tile([128, 40], f32, tag="Z")
                rz = rp.tile([128, 40], f32, tag="rz")
                ones_sb = rp.tile([128, C], f32r, tag="ones")
                Wcf_sb = rp.tile([128, CCH * E * GL * D], f32, tag="Wcf")
                Wcf2_sb = rp.tile([128, CCH * GL * D * E], f32, tag="Wcf2")
                mT8 = rp.tile([8, 2 * GL * CP], f32, tag="mT8")
                m_sb = rp.tile([128, CCH * 2 * GL * D], f32, tag="m_sb")
                s_sb = rp.tile([128, CCH * 2 * E], f32, tag="s_sb")
                sf_sb = rp.tile([128, CCH * 2 * E], f32, tag="sf_sb")
                vtmp = rp.tile([128, CCH * 2 * E], f32, tag="vtmp")
                sqv = rp.tile([128, 6], f32, tag="sqv")
                vp1 = rp.tile([128, 6], f32, tag="vp1")
                vp2 = rp.tile([128, 6], f32, tag="vp2")
                scale_v = rp.tile([128, 6], f32, tag="scale_v")
                v_sb = rp.tile([128, CCH * 2 * E], f32, tag="v_sb")
                wv_c = rp.tile([128, CCH * 2 * GL * D], f32, tag="wv_c")
                wv_dc = rp.tile([8, 2 * GL * CP], f32r, tag="wv_dc")
                sv = rp.tile([128, 6], f32, tag="sv")
                onorm = rp.tile([128, 6], f32, tag="onorm")

                nc.sync.dma_start(bias8_sb[:], bias8)
                nc.sync.dma_start(Wcf_sb[:], Wcf)
                nc.sync.dma_start(Wcf2_sb[:], Wcf2)

                # receive own 72-column window from each rank's block:
                # stage8[d, q*576 + occ*288 + t'*18 + j] +=
                #   ag_out[(4b+2occ+icc)*128 + t'*8 + d, 72k + gl*18 + j]
                stageA = rs.tile([8, Q * S], f32, tag="stageA")
                stageB = rs.tile([8, Q * S], f32, tag="stageB")
                rk = nc.sync.alloc_register("colbase")
                nc.sync.reg_load(rk, gmeta_sb[0:1, 1:2])
                rkv = nc.sync.snap(rk, donate=True, min_val=0, max_val=504)
                for b in range(2):
                    for occ in range(2):
                        for icc in range(2):
                            m = 4 * b + 2 * occ + icc
                            dstt = stageB if icc else stageA
                            base = b * 2304 + occ * 1152
                            for d in range(8):
                                sap = bass.AP(
                                    tensor=ag_out.tensor,
                                    offset=rkv + m * 73728 + d * 576,
                                    ap=[[18, 4], [8 * 576, 16], [1, 18]],
                                    dep_tracking_offset=0)
                                nc.sync.dma_start(
                                    dstt[d:d + 1, base:base + 1152],
                                    sap)
                # sum icc partials while permuting (occ,gl,.) -> (gl,occ,.)
                for b in range(2):
                    nc.vector.tensor_add(
                        stage8[:, b * 2304:(b + 1) * 2304].rearrange(
                            "p (g o x) -> p g o x", g=4, o=2),
                        stageA[:, b * 2304:(b + 1) * 2304].rearrange(
                            "p (o g x) -> p g o x", o=2, g=4),
                        stageB[:, b * 2304:(b + 1) * 2304].rearrange(
                            "p (o g x) -> p g o x", o=2, g=4))
                nc.vector.tensor_add(stage8[:], stage8[:], bias8_sb[:])

                # transpose to [p, d] layout (640-padded blocks, zero pad)
                nc.vector.memset(h2T[:], 0.0)
                h2Tv = h2T[:].rearrange("p (q c d) -> p q c d", q=Q, c=PCH)
                st_v = stage8[:].rearrange("p (q r) -> p q r", q=Q)
                for q in range(Q):
                    tp = ps_m.tile([128, PCH * 8], f32, tag="mT")
                    for pch in range(PCH):
                        rows = 128 if pch < 4 else 64
                        nc.tensor.transpose(
                            tp[0:rows, pch * 8:(pch + 1) * 8],
                            st_v[0:8, q, pch * 128:pch * 128 + rows],
                            ident[0:8, 0:8])
                    nc.scalar.copy(h2Tv[:, q, 0:4, :], tp[:, 0:32])
                    nc.scalar.copy(h2Tv[0:64, q, 4, :], tp[0:64, 32:40])

                # squash: h2T <- h2T * sq/((1+sq)*sqrt(sq+1e-8))
                nc.vector.tensor_mul(tmp320[:], h2T[:], h2T[:])
                nc.vector.tensor_reduce(
                    sq[:], tmp320[:].rearrange("p (g d) -> p g d", d=D),
                    axis=AX.X, op=ALU.add)
                nc.scalar.activation(sp1[:], sq[:], AF.Sqrt, bias=epsc[:, 0:1])
                nc.vector.tensor_scalar_add(sp2[:], sq[:], 1.0)
                nc.vector.tensor_mul(sp1[:], sp1[:], sp2[:])
                nc.vector.reciprocal(sp1[:], sp1[:])
                nc.vector.tensor_mul(scale[:], sq[:], sp1[:])
                nc.vector.tensor_mul(
                    h2T[:].rearrange("p (g d) -> p g d", d=D),
                    h2T[:].rearrange("p (g d) -> p g d", d=D),
                    scale[:].rearrange("p (g o) -> p g o", o=1)
                    .broadcast_to([128, 40, D]))

                # primT[d, q*640 + r] = stage8[d, r*8+q] * scale[r]
                # (scale transposed+flattened to (q, pch, p) order, then
                #  partition-broadcast to the 8 d-rows via DMA)
                tsc = ps_t.tile([40, 128], f32, tag="tmix")
                nc.tensor.transpose(tsc[:], scale[:], ident[:])
                nc.vector.tensor_copy(sct[:], tsc[:])
                nc.sync.dma_start(scale_flat[:], sct[:])
                for d in range(8):
                    nc.sync.dma_start(scale8[d:d + 1, :], scale_flat[0:1, :])
                nc.vector.memset(primT[:].bitcast(f32), 0.0)
                nc.vector.tensor_mul(
                    primT[:].rearrange("p (q r) -> p q r", r=640)[:, :, 0:S],
                    stage8[:].rearrange("p (q r) -> p q r", q=Q),
                    scale8[:].rearrange("p (q r) -> p q r", r=640)[:, :, 0:S])

                nc.vector.memset(ones_sb[:].bitcast(f32), 1.0)
                nc.vector.memset(mT8[:], 0.0)

                rs.release()
                rl = rctx.enter_context(tc.tile_pool(name="rl", bufs=1))
                b_log = rl.tile([128, Q * PCH * C], f32, tag="b_log")
                expb = rl.tile([128, Q * PCH * C], f32r, tag="expb")

                b_lv = b_log[:].rearrange("p (q c x) -> p q c x", q=Q, c=PCH)
                e_lv = expb[:].rearrange("p (q c x) -> p q c x", q=Q, c=PCH)
                psc = ps_sc[:].rearrange("p (g d) -> p g d", d=D)
                Wcfv = Wcf_sb[:].rearrange("p (c e g d) -> p c e g d",
                                           c=CCH, e=E, g=GL)
                Wcf2v = Wcf2_sb[:].rearrange("p (c g d e) -> p c g d e",
                                             c=CCH, g=GL, d=D)
                m_v = m_sb[:].rearrange("p (c b x) -> p c b x", c=CCH, b=2)
                s_v = s_sb[:].rearrange("p (c b e) -> p c b e", c=CCH, b=2)
                sf_v = sf_sb[:].rearrange("p (c b e) -> p c b e", c=CCH, b=2)
                v_v = v_sb[:].rearrange("p (c b e) -> p c b e", c=CCH, b=2)
                wv_v = wv_c[:].rearrange("p (c b x) -> p c b x", c=CCH, b=2)

                for it in range(3):
                    # --- coupling coefficients -> scaled prim rows
                    if it == 0:
                        nc.scalar.mul(ps_sc[:], h2T[:], 1.0 / C)
                    else:
                        for q in range(Q):
                            nc.scalar.activation(
                                e_lv[:, q, :, :].rearrange(
                                    "p c x -> p (c x)"),
                                b_lv[:, q, :, :].rearrange(
                                    "p c x -> p (c x)"),
                                AF.Exp, bias=zeroc[:, 0:1])
                            nc.vector.tensor_reduce(
                                Zt[:, q * PCH:(q + 1) * PCH],
                                e_lv[:, q, :, :], axis=AX.X, op=ALU.add)
                        nc.vector.reciprocal(rz[:], Zt[:])
                        nc.vector.tensor_mul(
                            psc,
                            h2T[:].rearrange("p (g d) -> p g d", d=D),
                            rz[:].rearrange("p (g o) -> p g o", o=1)
                            .broadcast_to([128, 40, D]))

                    # --- m^T[d, c] per (b, group):  prim_scaled.T @ exp
                    for q in range(Q):
                        mp = ps_m.tile([8, C], f32, tag="mT")
                        for pch in range(PCH):
                            rhs = (ones_sb[:] if it == 0
                                   else e_lv[:, q, pch, :])
                            nc.tensor.matmul(mp[:],
                                             psc[:, q * PCH + pch, :],
                                             rhs, start=(pch == 0),
                                             stop=(pch == PCH - 1))
                        nc.scalar.copy(mT8[0:8, q * CP:q * CP + C], mp[:])

                    # --- transpose m to class-partitioned layout
                    for b in range(2):
                        for ch in range(CCH):
                            tp = ps_t.tile([128, 32], f32, tag="tmix")
                            for g in range(GL):
                                q = b * GL + g
                                nc.tensor.transpose(
                                    tp[:, g * D:(g + 1) * D],
                                    mT8[0:8, q * CP + ch * 128:
                                        q * CP + (ch + 1) * 128],
                                    ident[0:8, 0:8])
                            nc.vector.tensor_copy(m_v[:, ch, b, :], tp[:])

                    # --- class votes s[c, e] = sum_{g,d} m * W
                    for b in range(2):
                        for ch in range(CCH):
                            st = rp.tile([128, E * GL * D], f32, tag="stmp")
                            nc.vector.tensor_mul(
                                st[:].rearrange("p (e g d) -> p e g d",
                                                e=E, g=GL),
                                Wcfv[:, ch],
                                m_v[:, ch, b, :]
                                .rearrange("p (o g d) -> p o g d", o=1, g=GL)
                                .broadcast_to([128, E, GL, D]))
                            nc.vector.tensor_reduce(
                                s_v[:, ch, b, :],
                                st[:].rearrange("p (e x) -> p e x", e=E),
                                axis=AX.X, op=ALU.add)

                    nc.sync.dma_start(s_in[:], s_sb[:])
                    nc.gpsimd.collective_compute(
                        "AllReduce", ALU.add,
                        replica_groups=[list(range(NC))],
                        ins=[s_in[:].opt()], outs=[s_out[:].opt()])
                    nc.sync.dma_start(sf_sb[:], s_out[:])

                    # --- v = squash(s) pieces
                    nc.vector.tensor_mul(vtmp[:], sf_sb[:], sf_sb[:])
                    nc.vector.tensor_reduce(
                        sqv[:], vtmp[:].rearrange("p (g e) -> p g e", e=E),
                        axis=AX.X, op=ALU.add)
                    nc.scalar.activation(vp1[:], sqv[:], AF.Sqrt, bias=epsc[:, 0:1])
                    nc.vector.tensor_scalar_add(vp2[:], sqv[:], 1.0)
                    nc.vector.tensor_mul(vp1[:], vp1[:], vp2[:])
                    nc.vector.reciprocal(vp1[:], vp1[:])
                    nc.vector.tensor_mul(scale_v[:], sqv[:], vp1[:])

                    if it < 2:
                        nc.vector.tensor_mul(
                            v_sb[:].rearrange("p (g e) -> p g e", e=E),
                            sf_sb[:].rearrange("p (g e) -> p g e", e=E),
                            scale_v[:].rearrange("p (g o) -> p g o", o=1)
                            .broadcast_to([128, 6, E]))
                        # wv[c, (g,d)] = sum_e W2 * v
                        for b in range(2):
                            for ch in range(CCH):
                                wt = rp.tile([128, GL * D * E], f32,
                                             tag="wtmp")
                                nc.vector.tensor_mul(
                                    wt[:].rearrange(
                                        "p (g d e) -> p g d e", g=GL, d=D),
                                    Wcf2v[:, ch],
                                    v_v[:, ch, b, :]
                                    .rearrange("p (o u e) -> p o u e",
                                               o=1, u=1)
                                    .broadcast_to([128, GL, D, E]))
                                nc.vector.tensor_reduce(
                                    wv_v[:, ch, b, :],
                                    wt[:].rearrange("p (x e) -> p x e", e=E),
                                    axis=AX.X, op=ALU.add)
                        # transpose wv to [d, c]
                        for b in range(2):
                            for ch in range(CCH):
                                tp = ps_t.tile([8, 512], f32, tag="tmix")
                                for g in range(GL):
                                    nc.tensor.transpose(
                                        tp[:, g * 128:(g + 1) * 128],
                                        wv_v[:, ch, b, g * D:(g + 1) * D],
                                        ident[:])
                                dst = wv_dc[0:8, :].rearrange(
                                    "p (q c x) -> p q c x", q=Q, c=CCH)[
                                    :, b * GL:(b + 1) * GL, ch, :]
                                nc.scalar.copy(dst, tp[:])
                        # b_log += prim @ wv  (5 pch chunks -> 5 psum
                        # banks of one tile, single merged evac per block)
                        for q in range(Q):
                            up = ps_u.tile([128, PCH * 512], f32, tag="u")
                            for pch in range(PCH):
                                nc.tensor.matmul(
                                    up[:, pch * 512:pch * 512 + C],
                                    primT[0:8, q * 640 + pch * 128:
                                          q * 640 + (pch + 1) * 128],
                                    wv_dc[0:8, q * CP:q * CP + C],
                                    start=True, stop=True)
                            upv = up[:].rearrange(
                                "p (c x) -> p c x", c=PCH)[:, :, 0:C]
                            if it == 0:
                                nc.scalar.copy(b_lv[:, q, :, :], upv)
                            else:
                                nc.vector.tensor_add(
                                    b_lv[:, q, :, :], b_lv[:, q, :, :], upv)
                    else:
                        # output ||v|| = sqrt(sqv) * scale_v
                        nc.scalar.activation(sv[:], sqv[:], AF.Sqrt,
                                             bias=zeroc[:, 0:1])
                        nc.vector.tensor_mul(onorm[:], sv[:], scale_v[:])
                        ov = onorm[:].rearrange("p (g b) -> p g b", b=2)
                        for b in range(2):
                            nc.sync.dma_start(
                                out[b:b + 1, 0:256].rearrange(
                                    "o (ch p) -> o p ch", p=128),
                                ov[:, 0:2, b])
                            nc.sync.dma_start(out[b:b + 1, 256:276],
                                              ov[0:20, 2, b])

    nc.compile()
    return nc


def _host_prep(x, conv1_w, conv1_b, conv2_w, conv2_b, W):
    """Build the 8 per-core input maps."""
    x = np.asarray(x, np.float32)
    w1T_full = np.asarray(conv1_w, np.float32).reshape(256, 81).T.copy()
    w2 = np.asarray(conv2_w, np.float32).reshape(256, 256, 81)
    # [ic, pos, oc]
    w2T_full = np.ascontiguousarray(w2.transpose(1, 2, 0))
    conv1_b = np.asarray(conv1_b, np.float32)
    conv2_b = np.asarray(conv2_b, np.float32)
    W = np.asarray(W, np.float32)  # [32, 276, 8, 16]

    # bias8[d, q*576 + t*18 + j] = conv2_b[t*8 + d]  (same for every q)
    bias_blk = np.zeros((8, S), np.float32)
    for t in range(NT):
        for d in range(D):
            bias_blk[d, t * 18:(t + 1) * 18] = conv2_b[t * 8 + d]
    bias8 = np.tile(bias_blk, (1, Q))

    in_maps = []
    for k in range(NC):
        b_k, occ_k, icc_k = k >> 2, (k >> 1) & 1, k & 1
        xb = np.ascontiguousarray(x[b_k, 0])
        w1T = np.ascontiguousarray(w1T_full[:, icc_k * 128:(icc_k + 1) * 128])
        b1 = conv1_b[icc_k * 128:(icc_k + 1) * 128].reshape(128, 1).copy()
        w2T = np.ascontiguousarray(
            w2T_full[icc_k * 128:(icc_k + 1) * 128, :,
                     occ_k * 128:(occ_k + 1) * 128]).reshape(128, 81 * 128)

        gmeta = np.zeros((1, 16), np.int32)
        gmeta[0, 1] = 72 * k             # own column window in ag_out

        # Wcf[p, ch, e, g_l, d] / Wcf2[p, ch, g_l, d, e] = W[4k+g_l, c, d, e]
        Wk = W[4 * k:4 * k + 4]  # [GL, 276, 8, 16]
        Wp = np.zeros((GL, CP, D, E), np.float32)
        Wp[:, :C] = Wk
        Wp = Wp.reshape(GL, CCH, 128, D, E)
        Wcf = np.ascontiguousarray(
            Wp.transpose(2, 1, 4, 0, 3)).reshape(128, -1)  # p,ch,e,g,d
        Wcf2 = np.ascontiguousarray(
            Wp.transpose(2, 1, 0, 3, 4)).reshape(128, -1)  # p,ch,g,d,e

        in_maps.append({
            "xb": xb, "w1T": w1T, "b1": b1, "w2T": w2T,
            "bias8": bias8, "gmeta": gmeta, "Wcf": Wcf, "Wcf2": Wcf2,
        })
    return in_maps


def kernel(x, conv1_w, conv1_b, conv2_w, conv2_b, W):
    if "nc" not in _CACHE:
        _CACHE["nc"] = _build_program()
    nc = _CACHE["nc"]
    in_maps = _host_prep(x, conv1_w, conv1_b, conv2_w, conv2_b, W)

    from concourse.bass_utils import run_bass_kernel_spmd
    res = run_bass_kernel_spmd(nc, in_maps, core_ids=list(range(NC)),
                               trace=bool(int(os.environ.get(
                                   "CAPS_TRACE", "0"))))
    _CACHE["last_result"] = res
    return np.asarray(res.results[0]["out"], np.float32)



# revision 29
# speedup vs baseline: 1.4194x; 1.4194x over previous
"""CapsNet (nn_CapsNet_25194278158439) Trainium2 Bass kernel, 8-core SPMD.

Math (faithful to the reference, including its W-tiling quirk):
  conv1 (9x9 valid) + relu -> conv2 (9x9 stride2) + bias -> primary capsules
  prim[b, p, d],  p = t*576 + s  (t = capsule type 0..31, s = spatial 0..575)
  u_hat[b,p,c,:] = prim[b,p,:] @ W[s mod 32]
  3 rounds of routing-by-agreement over C=276 classes; out = ||v||  [2, 276].

Key factorizations (u_hat and b_log never materialized):
  s_vec[b,c,e] = sum_g sum_d m[b,g,c,d] * W[g,c,d,e]
  m[b,g,c,d]   = sum_{p in g} softmax_c(b)[b,p,c] * prim[b,p,d]   (matmul)
  b[p,c]       = prim[p,:] . A[g(p),c,:],  A = W[g] @ (v0+v1+...)  (matmul)

Sharding (8 cores = batch x oc-half x ic-half for conv):
  conv2 partials redistributed with ONE AllToAll of the per-rank 72-column
  group windows (bf16, 147KB) -- each routing core owns weight groups
  4k..4k+3 (4608 rows x 276 classes).  Each routing iteration does one
  bf16 AllReduce of the class vote s (25KB, ~5us).
  Mostly-bf16 datapath; all activation functions from one Act table set
  (ln+exp used for sqrt = exp(0.5*ln(x))) to avoid table reloads.
"""

import os
import numpy as np

NC = 8          # cores
C = 276         # classes
D = 8           # primary capsule dim
E = 16          # digit capsule dim
NT = 32         # capsule types
S = 576         # spatial positions per type (24*24)
GL = 4          # weight groups per core
Q = 8           # row blocks per core: (batch, local group)
PCH = 5         # 128-row chunks per 640-padded block
CP = 384        # class dim padded to 3*128
CCH = 3         # class chunks

_CACHE = {}


def _build_program():
    import concourse.bass as bass
    import concourse.mybir as mybir
    import concourse.tile as tile
    from concourse import bacc
    from concourse.masks import make_identity

    f32 = mybir.dt.float32
    bf16 = mybir.dt.bfloat16
    AX = mybir.AxisListType
    AF = mybir.ActivationFunctionType
    ALU = mybir.AluOpType

    nc = bacc.Bacc("TRN2", target_bir_lowering=False, debug=False,
                   num_devices=NC)

    # ---- kernel I/O -------------------------------------------------------
    xb = nc.dram_tensor("xb", [64, 64], bf16, kind="ExternalInput").ap()
    w1T = nc.dram_tensor("w1T", [81, 128], bf16, kind="ExternalInput").ap()
    b1 = nc.dram_tensor("b1", [128, 1], f32, kind="ExternalInput").ap()
    w2T = nc.dram_tensor("w2T", [128, 81 * 128], bf16,
                         kind="ExternalInput").ap()
    b2 = nc.dram_tensor("b2", [128, 1], f32, kind="ExternalInput").ap()
    Wcf = nc.dram_tensor("Wcf", [128, CCH * E * GL * D], bf16,
                         kind="ExternalInput").ap()
    Wcf2 = nc.dram_tensor("Wcf2", [128, CCH * GL * D * E], bf16,
                          kind="ExternalInput").ap()
    out = nc.dram_tensor("out", [2, C], f32, kind="ExternalOutput").ap()

    lp = nc.allow_low_precision("bf16 kernel; graded at rel<2e-2")
    lp.__enter__()
    with tile.TileContext(nc) as tc:
        import contextlib
        with contextlib.ExitStack() as ctx:
            pool = ctx.enter_context(tc.tile_pool(name="const", bufs=1))
            dram = ctx.enter_context(tc.tile_pool(name="dram", bufs=1,
                                                  space="DRAM"))

            ident = pool.tile([128, 128], bf16, tag="ident")
            make_identity(nc, ident[:])
            identf = pool.tile([128, 128], f32, tag="identf")
            make_identity(nc, identf[:])
            epsc = pool.tile([128, 1], f32, tag="epsc")
            nc.vector.memset(epsc[:], 1e-8)

            # collective bounce buffers
            # chunk layout [p=(t',d), gl, j] with strides (76, 18, 1):
            # 76 (not 72) so DMA access patterns stay unmergeable/3-dim
            a2a_in = dram.tile([8, 9728], bf16, tag="a2a_in")
            a2a_out = dram.tile([8, 9728], bf16, tag="a2a_out")
            s_in = dram.tile([128, 96], bf16, tag="s_in")
            s_out = dram.tile([128, 96], bf16, tag="s_out")
            warm_in = dram.tile([8, 16], f32, tag="warm_in")
            warm_out = dram.tile([8, 16], f32, tag="warm_out")

            # warmup collectives: absorb any CC channel setup while conv runs
            warm_sb = pool.tile([8, 16], f32, tag="warm_sb")
            nc.vector.memset(warm_sb[:], 0.0)
            nc.sync.dma_start(warm_in[:], warm_sb[:])
            nc.gpsimd.collective_compute(
                "AllToAll", ALU.bypass, replica_groups=[list(range(NC))],
                ins=[warm_in[:].opt()], outs=[warm_out[:].opt()])
            nc.gpsimd.collective_compute(
                "AllReduce", ALU.add, replica_groups=[list(range(NC))],
                ins=[warm_in[:].opt()], outs=[warm_out[:].opt()])

            # ============ conv phase ======================================
            with contextlib.ExitStack() as cctx:
                cpool = cctx.enter_context(tc.tile_pool(name="conv", bufs=1))
                ps1 = cctx.enter_context(
                    tc.tile_pool(name="ps1", bufs=2, space="PSUM"))
                ps2 = cctx.enter_context(
                    tc.tile_pool(name="ps2", bufs=1, space="PSUM"))

                w1T_sb = cpool.tile([81, 128], bf16, tag="w1T")
                b1_sb = cpool.tile([128, 1], f32, tag="b1")
                b2_sb = cpool.tile([128, 1], f32, tag="b2")
                patches = cpool.tile([81, 3136], bf16, tag="patches")
                h1 = cpool.tile([128, 3136], bf16, tag="h1")
                w2T_sb = cpool.tile([128, 81 * 128], bf16, tag="w2T")
                h2p3 = cpool.tile([128, 576], bf16, tag="h2p3")

                nc.sync.dma_start(w1T_sb[:], w1T)
                nc.sync.dma_start(b1_sb[:], b1)
                nc.sync.dma_start(b2_sb[:], b2)

                # conv1 im2col: patches[(kh,kw), (oh,ow)] = x[oh+kh, ow+kw]
                for kh in range(9):
                    src = bass.AP(tensor=xb.tensor, offset=kh * 64,
                                  ap=[[1, 9], [64, 56], [1, 56]])
                    nc.sync.dma_start(
                        patches[kh * 9:(kh + 1) * 9, :].rearrange(
                            "p (a b) -> p a b", a=56), src)

                # conv2 weights: 4 chunked DMAs across two queue engines
                for ci in range(4):
                    eng = nc.gpsimd if ci % 2 else nc.sync
                    eng.dma_start(
                        w2T_sb[:, ci * 2592: (ci + 1) * 2592],
                        w2T[:, ci * 2592: (ci + 1) * 2592])

                # conv1: h1[oc, s] = relu(w1.T @ patches + b1)
                for j in range(7):
                    pt = ps1.tile([128, 448], f32, tag="c1")
                    nc.tensor.matmul(pt[:], w1T_sb[:],
                                     patches[:, j * 448:(j + 1) * 448],
                                     start=True, stop=True)
                    nc.scalar.activation(h1[:, j * 448:(j + 1) * 448], pt[:],
                                         AF.Relu, bias=b1_sb[:, 0:1])

                # conv2: 81-position accumulation, stride 2
                psA = ps2.tile([128, 288], f32, tag="psA")
                psB = ps2.tile([128, 288], f32, tag="psB")
                hv = h1[:].rearrange("p (h w) -> p h w", w=56)
                for pos in range(81):
                    kh, kw = divmod(pos, 9)
                    vh = hv.rearrange("p (oh two) w -> p oh two w", two=2)[
                        :, kh // 2: kh // 2 + 24, kh % 2, :]
                    vw = vh.rearrange("p oh (ow two) -> p oh ow two", two=2)[
                        :, :, kw // 2: kw // 2 + 24, kw % 2]
                    lhsT = w2T_sb[:, pos * 128:(pos + 1) * 128]
                    nc.tensor.matmul(psA[:], lhsT, vw[:, 0:12, :],
                                     start=(pos == 0), stop=(pos == 80))
                    nc.tensor.matmul(psB[:], lhsT, vw[:, 12:24, :],
                                     start=(pos == 0), stop=(pos == 80))
                # evac + conv2 bias (zero on icc=1 cores) + permute to
                # group-major: h2p3[p, g*18 + j] = psum[p, s=j*32+g] + b2
                nc.scalar.activation(
                    h2p3[:].rearrange("p (g j) -> p g j", g=32)[:, :, 0:9],
                    psA[:].rearrange("p (j g) -> p g j", g=32),
                    AF.Identity, bias=b2_sb[:, 0:1])
                nc.scalar.activation(
                    h2p3[:].rearrange("p (g j) -> p g j", g=32)[:, :, 9:18],
                    psB[:].rearrange("p (j g) -> p g j", g=32),
                    AF.Identity, bias=b2_sb[:, 0:1])

                # send buffer: chunk r = our columns for rank r, element
                # (p=(t',d), r, gl, j) at chunk offset p*76 + gl*18 + j
                zpad = cpool.tile([128, 32], bf16, tag="zpad")
                nc.vector.memset(zpad[:], 0.0)
                nc.sync.dma_start(
                    bass.AP(tensor=a2a_in.tensor, offset=72,
                            ap=[[76, 128], [9728, 8], [1, 4]]),
                    zpad[:].rearrange("p (r x) -> p r x", r=8))
                for gl in range(4):
                    dst = bass.AP(
                        tensor=a2a_in.tensor, offset=gl * 18,
                        ap=[[76, 128], [9728, 8], [1, 18]])
                    src = h2p3[:].rearrange(
                        "p (r g j) -> p r g j", r=8, g=4)[:, :, gl, :].opt()
                    nc.sync.dma_start(dst, src)

            nc.gpsimd.collective_compute(
                "AllToAll", ALU.bypass,
                replica_groups=[list(range(NC))],
                ins=[a2a_in[:].opt()], outs=[a2a_out[:].opt()])

            # ============ routing phase ===================================
            with contextlib.ExitStack() as rctx:
                rp = rctx.enter_context(tc.tile_pool(name="rt", bufs=1))
                # PSUM budget (8 banks): ub1 3 + ub2 2 + mp 1 + scratch 2
                ps_t = rctx.enter_context(
                    tc.tile_pool(name="pst", bufs=2, space="PSUM"))
                ps_m = rctx.enter_context(
                    tc.tile_pool(name="psm", bufs=1, space="PSUM"))
                ps_u = rctx.enter_context(
                    tc.tile_pool(name="psu", bufs=1, space="PSUM"))
                expp = rctx.enter_context(tc.tile_pool(name="expp", bufs=2))
                stp = rctx.enter_context(tc.tile_pool(name="stp", bufs=2))

                def scratch():
                    # one generic 1-bank psum tile; callers slice/bitcast
                    scr = ps_t.tile([128, 256], f32, tag="scr", name="scr")
                    return scr

                stageA = rp.tile([8, Q * S], bf16, tag="stageA")
                stageB = rp.tile([8, Q * S], bf16, tag="stageB")
                stage8 = rp.tile([8, Q * S], bf16, tag="stage8")
                h2T = rp.tile([128, Q * PCH * D], bf16, tag="h2T")
                h2Ts = rp.tile([128, Q * PCH * D], bf16, tag="h2Ts")
                tmp320 = rp.tile([128, Q * PCH * D], f32, tag="tmp320")
                sq = rp.tile([128, 40], f32, tag="sq")
                sp1 = rp.tile([128, 40], f32, tag="sp1")
                sp2 = rp.tile([128, 40], f32, tag="sp2")
                scale = rp.tile([128, 40], f32, tag="scale")
                sct = rp.tile([40, 128], f32, tag="sct")
                scale_flat = dram.tile([1, 5120], f32, tag="scale_flat")
                scale8 = rp.tile([8, 5120], f32, tag="scale8")
                primT = rp.tile([8, Q * 640], bf16, tag="primT")
                psc = rp.tile([128, Q * PCH * D], bf16, tag="psc")
                Zt = rp.tile([128, 40], bf16, tag="Zt")
                rz = rp.tile([128, 40], f32, tag="rz")
                ones_sb = rp.tile([128, C], bf16, tag="ones")
                Wcf_sb = rp.tile([128, CCH * E * GL * D], bf16, tag="Wcf")
                Wcf2_sb = rp.tile([128, CCH * GL * D * E], bf16, tag="Wcf2")
                mT8 = rp.tile([8, Q * CP], f32, tag="mT8")
                m_cm = rp.tile([128, CCH * 2 * GL * D], bf16, tag="m_cm")
                s_sb = rp.tile([128, CCH * 2 * E], bf16, tag="s_sb")
                sf_sb = rp.tile([128, CCH * 2 * E], bf16, tag="sf_sb")
                vtmp = rp.tile([128, CCH * 2 * E], f32, tag="vtmp")
                sqv = rp.tile([128, 6], f32, tag="sqv")
                vp1 = rp.tile([128, 6], f32, tag="vp1")
                vp2 = rp.tile([128, 6], f32, tag="vp2")
                scale_v = rp.tile([128, 6], f32, tag="scale_v")
                v_sb = rp.tile([128, CCH * 2 * E], bf16, tag="v_sb")
                wv_c = rp.tile([128, CCH * 2 * GL * D], bf16, tag="wv_c")
                A_cm = rp.tile([128, CCH * 2 * GL * D], bf16, tag="A_cm")
                A_dc = rp.tile([8, Q * CP], bf16, tag="A_dc")
                sv = rp.tile([128, 6], f32, tag="sv")
                onorm = rp.tile([128, 6], f32, tag="onorm")

                nc.sync.dma_start(Wcf_sb[:], Wcf)
                nc.sync.dma_start(Wcf2_sb[:], Wcf2)
                nc.vector.memset(ones_sb[:], 1.0)
                nc.vector.memset(h2T[:], 0.0)
                nc.vector.memset(primT[:], 0.0)
                nc.vector.memset(
                    mT8[0:8, :].rearrange("p (q c) -> p q c", q=Q)[
                        :, :, C:CP], 0.0)

                # receive: stage[d, q(b,gl)*576 + occ*288 + t'*18 + j]
                #   = a2a_out chunk m at (t'*8+d)*76 + gl*18 + j
                engs = [nc.sync, nc.gpsimd, nc.scalar, nc.sync]
                for m in range(8):
                    b_, occ, icc = m >> 2, (m >> 1) & 1, m & 1
                    dstt = stageB if icc else stageA
                    for gl in range(4):
                        dst = dstt[:].rearrange(
                            "p (b g o n) -> p b g o n",
                            b=2, g=4, o=2, n=288)[:, b_, gl, occ]
                        src = bass.AP(
                            tensor=a2a_out.tensor,
                            offset=m * 9728 + gl * 18,
                            ap=[[76, 8], [8 * 76, 16], [1, 18]])
                        engs[(4 * m + gl) % 4].dma_start(dst.opt(), src)
                nc.vector.tensor_add(stage8[:], stageA[:], stageB[:])

                # transpose to row-major [p, d] (640-padded blocks)
                h2Tv = h2T[:].rearrange("p (q c d) -> p q c d", q=Q, c=PCH)
                st_v = stage8[:].rearrange("p (q r) -> p q r", q=Q)
                for q in range(Q):
                    tp = scratch()[:].bitcast(bf16)
                    for pch in range(PCH):
                        rows = 128 if pch < 4 else 64
                        nc.tensor.transpose(
                            tp[0:rows, pch * 8:(pch + 1) * 8],
                            st_v[0:8, q, pch * 128:pch * 128 + rows],
                            ident[0:8, 0:8])
                    nc.scalar.copy(h2Tv[:, q, 0:4, :], tp[:, 0:32])
                    nc.scalar.copy(h2Tv[0:64, q, 4, :], tp[0:64, 32:40])

                # squash scale: scale = sq/((1+sq)*sqrt(sq+1e-8)),
                # sqrt via exp(0.5*ln(x)) to stay in one Act table set
                nc.scalar.activation(tmp320[:], h2T[:], AF.Square)
                nc.vector.tensor_reduce(
                    sq[:], tmp320[:].rearrange("p (g d) -> p g d", d=D),
                    axis=AX.X, op=ALU.add)
                nc.scalar.activation(sp1[:], sq[:], AF.Ln, bias=epsc[:, 0:1])
                nc.scalar.activation(sp1[:], sp1[:], AF.Exp, scale=0.5)
                nc.vector.tensor_scalar_add(sp2[:], sq[:], 1.0)
                nc.vector.tensor_mul(sp1[:], sp1[:], sp2[:])
                nc.vector.reciprocal(sp1[:], sp1[:])
                nc.vector.tensor_mul(scale[:], sq[:], sp1[:])
                nc.vector.tensor_mul(
                    h2Ts[:].rearrange("p (g d) -> p g d", d=D),
                    h2T[:].rearrange("p (g d) -> p g d", d=D),
                    scale[:].rearrange("p (g o) -> p g o", o=1)
                    .broadcast_to([128, 40, D]))

                # primT[d, q*640 + r] = stage8[d, r] * scale[r]
                tsc = scratch()[0:40, 0:128]
                nc.tensor.transpose(tsc, scale[:], identf[:])
                nc.vector.tensor_copy(sct[:], tsc)
                nc.sync.dma_start(scale_flat[:], sct[:])
                nc.sync.dma_start(
                    scale8[:],
                    bass.AP(tensor=scale_flat.tensor, offset=0,
                            ap=[[0, 8], [1, 5120]]))
                nc.vector.tensor_mul(
                    primT[:].rearrange("p (q r) -> p q r", r=640)[:, :, 0:S],
                    stage8[:].rearrange("p (q r) -> p q r", q=Q),
                    scale8[:].rearrange("p (q r) -> p q r", r=640)[:, :, 0:S])

                mv = m_cm[:].rearrange("p (c b x) -> p c b x", c=CCH, b=2)
                Wcfv = Wcf_sb[:].rearrange("p (c e g d) -> p c e g d",
                                           c=CCH, e=E, g=GL)
                Wcf2v = Wcf2_sb[:].rearrange("p (c g d e) -> p c g d e",
                                             c=CCH, g=GL, d=D)
                s_v = s_sb[:].rearrange("p (c b e) -> p c b e", c=CCH, b=2)
                sf_v = sf_sb[:].rearrange("p (c b e) -> p c b e", c=CCH, b=2)
                v_v = v_sb[:].rearrange("p (c b e) -> p c b e", c=CCH, b=2)
                wv_v = wv_c[:].rearrange("p (c b x) -> p c b x", c=CCH, b=2)
                A_v = A_cm[:].rearrange("p (c b x) -> p c b x", c=CCH, b=2)
                pscv = psc[:].rearrange("p (g d) -> p g d", d=D)
                h2Tsv = h2Ts[:].rearrange("p (g d) -> p g d", d=D)

                def emit_m(it):
                    """mT8[d, q*CP+c] = sum_rows psc[row,d] * rhs[row,c]."""
                    for q in range(Q):
                        exq = None
                        if it > 0:
                            # b = primT.T @ A_dc  -> exp -> Z -> psc
                            exq = expp.tile([128, PCH * C], bf16, tag="expb")
                            ub1 = ps_u.tile([128, 3 * 512], f32, tag="ub1")
                            ub2 = ps_u.tile([128, 2 * 512], f32, tag="ub2")
                            for pch in range(PCH):
                                ub = ub1 if pch < 3 else ub2
                                off = (pch if pch < 3 else pch - 3) * 512
                                nc.tensor.matmul(
                                    ub[:, off:off + C],
                                    primT[0:8, q * 640 + pch * 128:
                                          q * 640 + (pch + 1) * 128],
                                    A_dc[0:8, q * CP:q * CP + C],
                                    start=True, stop=True)
                            nc.scalar.activation(
                                exq[:, 0:3 * C],
                                ub1[:].rearrange(
                                    "p (c x) -> p c x", c=3)[:, :, 0:C],
                                AF.Exp)
                            nc.scalar.activation(
                                exq[:, 3 * C:PCH * C],
                                ub2[:].rearrange(
                                    "p (c x) -> p c x", c=2)[:, :, 0:C],
                                AF.Exp)
                            nc.vector.tensor_reduce(
                                Zt[:, q * PCH:(q + 1) * PCH],
                                exq[:].rearrange("p (c x) -> p c x", c=PCH),
                                axis=AX.X, op=ALU.add)
                            nc.vector.reciprocal(
                                rz[:, q * PCH:(q + 1) * PCH],
                                Zt[:, q * PCH:(q + 1) * PCH])
                            nc.vector.tensor_mul(
                                pscv[:, q * PCH:(q + 1) * PCH, :],
                                h2Tsv[:, q * PCH:(q + 1) * PCH, :],
                                rz[:, q * PCH:(q + 1) * PCH]
                                .rearrange("p (g o) -> p g o", o=1)
                                .broadcast_to([128, PCH, D]))
                        mp = ps_m.tile([8, C], f32, tag="mp")
                        for pch in range(PCH):
                            rhs = (ones_sb[:] if it == 0 else
                                   exq[:, pch * C:(pch + 1) * C])
                            nc.tensor.matmul(mp[:],
                                             psc[:, (q * PCH + pch) * D:
                                                 (q * PCH + pch + 1) * D],
                                             rhs, start=(pch == 0),
                                             stop=(pch == PCH - 1))
                        nc.scalar.copy(mT8[0:8, q * CP:q * CP + C], mp[:])

                def emit_s():
                    """class-major m, then s[c,e] = sum_{g,d} m*W."""
                    for b_ in range(2):
                        for ch in range(CCH):
                            tp = scratch()[:, 0:32]
                            for g in range(GL):
                                q = b_ * GL + g
                                nc.tensor.transpose(
                                    tp[:, g * D:(g + 1) * D],
                                    mT8[0:8, q * CP + ch * 128:
                                        q * CP + (ch + 1) * 128],
                                    identf[0:8, 0:8])
                            nc.vector.tensor_copy(mv[:, ch, b_, :], tp)
                    for b_ in range(2):
                        for ch in range(CCH):
                            stt = stp.tile([128, E * GL * D], bf16, tag="st")
                            nc.vector.tensor_mul(
                                stt[:].rearrange("p (e g d) -> p e g d",
                                                 e=E, g=GL),
                                Wcfv[:, ch],
                                mv[:, ch, b_, :]
                                .rearrange("p (o g d) -> p o g d", o=1, g=GL)
                                .broadcast_to([128, E, GL, D]))
                            nc.vector.tensor_reduce(
                                s_v[:, ch, b_, :],
                                stt[:].rearrange("p (e x) -> p e x", e=E),
                                axis=AX.X, op=ALU.add)

                def emit_squash_v():
                    nc.scalar.activation(vtmp[:], sf_sb[:], AF.Square)
                    nc.vector.tensor_reduce(
                        sqv[:], vtmp[:].rearrange("p (g e) -> p g e", e=E),
                        axis=AX.X, op=ALU.add)
                    nc.scalar.activation(vp1[:], sqv[:], AF.Ln,
                                         bias=epsc[:, 0:1])
                    nc.scalar.activation(vp1[:], vp1[:], AF.Exp, scale=0.5)
                    nc.vector.tensor_scalar_add(vp2[:], sqv[:], 1.0)
                    nc.vector.tensor_mul(vp1[:], vp1[:], vp2[:])
                    nc.vector.reciprocal(vp1[:], vp1[:])
                    nc.vector.tensor_mul(scale_v[:], sqv[:], vp1[:])

                for it in range(3):
                    if it == 0:
                        nc.vector.tensor_scalar_mul(psc[:], h2Ts[:], 1.0 / C)
                    emit_m(it)
                    emit_s()
                    nc.sync.dma_start(s_in[:], s_sb[:])
                    nc.gpsimd.collective_compute(
                        "AllReduce", ALU.add,
                        replica_groups=[list(range(NC))],
                        ins=[s_in[:].opt()], outs=[s_out[:].opt()])
                    nc.sync.dma_start(sf_sb[:], s_out[:])
                    emit_squash_v()

                    if it < 2:
                        nc.vector.tensor_mul(
                            v_sb[:].rearrange("p (g e) -> p g e", e=E),
                            sf_sb[:].rearrange("p (g e) -> p g e", e=E),
                            scale_v[:].rearrange("p (g o) -> p g o", o=1)
                            .broadcast_to([128, 6, E]))
                        # wv[c,(g,d)] = sum_e W2 * v ; A += wv
                        for b_ in range(2):
                            for ch in range(CCH):
                                wtt = stp.tile([128, GL * D * E], bf16,
                                               tag="wt")
                                nc.vector.tensor_mul(
                                    wtt[:].rearrange(
                                        "p (g d e) -> p g d e", g=GL, d=D),
                                    Wcf2v[:, ch],
                                    v_v[:, ch, b_, :]
                                    .rearrange("p (o u e) -> p o u e",
                                               o=1, u=1)
                                    .broadcast_to([128, GL, D, E]))
                                nc.vector.tensor_reduce(
                                    wv_v[:, ch, b_, :],
                                    wtt[:].rearrange("p (x e) -> p x e",
                                                     e=E),
                                    axis=AX.X, op=ALU.add)
                        if it == 0:
                            nc.vector.tensor_copy(A_cm[:], wv_c[:])
                        else:
                            nc.vector.tensor_add(A_cm[:], A_cm[:], wv_c[:])
                        # transpose A to [d, q*CP+c] for the b-matmul
                        for b_ in range(2):
                            for ch in range(CCH):
                                tpa = scratch()[:].bitcast(bf16)[0:8, :]
                                for g in range(GL):
                                    nc.tensor.transpose(
                                        tpa[:, g * 128:(g + 1) * 128],
                                        A_v[:, ch, b_, g * D:(g + 1) * D],
                                        ident[:])
                                dst = A_dc[0:8, :].rearrange(
                                    "p (q c x) -> p q c x", q=Q, c=CCH)[
                                    :, b_ * GL:(b_ + 1) * GL, ch, :]
                                nc.scalar.copy(dst, tpa[:])
                    else:
                        # output ||v|| = sqrt(sqv) * scale_v
                        nc.scalar.activation(sv[:], sqv[:], AF.Ln,
                                             bias=epsc[:, 0:1])
                        nc.scalar.activation(sv[:], sv[:], AF.Exp, scale=0.5)
                        nc.vector.tensor_mul(onorm[:], sv[:], scale_v[:])
                        ov = onorm[:].rearrange("p (g b) -> p g b", b=2)
                        for b_ in range(2):
                            nc.sync.dma_start(
                                out[b_:b_ + 1, 0:256].rearrange(
                                    "o (ch p) -> o p ch", p=128),
                                ov[:, 0:2, b_])
                            nc.sync.dma_start(out[b_:b_ + 1, 256:276],
                                              ov[0:20, 2, b_])

    lp.__exit__(None, None, None)
    nc.compile()
    return nc


def _to_bf16(x):
    import ml_dtypes
    return np.asarray(x, np.float32).astype(ml_dtypes.bfloat16)


def _host_prep(x, conv1_w, conv1_b, conv2_w, conv2_b, W):
    """Build the 8 per-core input maps."""
    x = np.asarray(x, np.float32)
    w1T_full = np.asarray(conv1_w, np.float32).reshape(256, 81).T.copy()
    w2 = np.asarray(conv2_w, np.float32).reshape(256, 256, 81)
    w2T_full = np.ascontiguousarray(w2.transpose(1, 2, 0))  # [ic, pos, oc]
    conv1_b = np.asarray(conv1_b, np.float32)
    conv2_b = np.asarray(conv2_b, np.float32)
    W = np.asarray(W, np.float32)  # [32, 276, 8, 16]

    in_maps = []
    for k in range(NC):
        b_k, occ_k, icc_k = k >> 2, (k >> 1) & 1, k & 1
        xb = _to_bf16(x[b_k, 0])
        w1T = _to_bf16(w1T_full[:, icc_k * 128:(icc_k + 1) * 128])
        b1 = conv1_b[icc_k * 128:(icc_k + 1) * 128].reshape(128, 1).copy()
        w2T = _to_bf16(
            w2T_full[icc_k * 128:(icc_k + 1) * 128, :,
                     occ_k * 128:(occ_k + 1) * 128]).reshape(128, 81 * 128)
        if icc_k == 0:
            b2 = conv2_b[occ_k * 128:(occ_k + 1) * 128].reshape(128, 1).copy()
        else:
            b2 = np.zeros((128, 1), np.float32)

        # Wcf[p, ch, e, g_l, d] / Wcf2[p, ch, g_l, d, e] = W[4k+g_l, c, d, e]
        Wk = W[4 * k:4 * k + 4]  # [GL, 276, 8, 16]
        Wp = np.zeros((GL, CP, D, E), np.float32)
        Wp[:, :C] = Wk
        Wp = Wp.reshape(GL, CCH, 128, D, E)
        Wcf = _to_bf16(np.ascontiguousarray(
            Wp.transpose(2, 1, 4, 0, 3)).reshape(128, -1))
        Wcf2 = _to_bf16(np.ascontiguousarray(
            Wp.transpose(2, 1, 0, 3, 4)).reshape(128, -1))

        in_maps.append({
            "xb": xb, "w1T": w1T, "b1": b1, "w2T": w2T, "b2": b2,
            "Wcf": Wcf, "Wcf2": Wcf2,
        })
    return in_maps


def kernel(x, conv1_w, conv1_b, conv2_w, conv2_b, W):
    if "nc" not in _CACHE:
        _CACHE["nc"] = _build_program()
    nc = _CACHE["nc"]
    in_maps = _host_prep(x, conv1_w, conv1_b, conv2_w, conv2_b, W)

    from concourse.bass_utils import run_bass_kernel_spmd
    res = run_bass_kernel_spmd(nc, in_maps, core_ids=list(range(NC)),
                               trace=bool(int(os.environ.get(
                                   "CAPS_TRACE", "0"))))
    _CACHE["last_result"] = res
    return np.asarray(res.results[0]["out"], np.float32)
